# revision 1
# baseline (speedup 1.0000x reference)
"""Trainium2 Bass kernel for LocationSensitiveAttention.

Math (B=64, T=2048, E=768, A=128, L=32, K=31):
    Ws   = decoder_state @ W_w + W_b                      [B, A]
    Vh   = encoding @ V_w                                 [B, T, A]
    loc  = conv1d(caw, F_w, pad=15)                       [B, L, T]
    Uf   = loc^T @ U_w                                    [B, T, A]
    e    = tanh(Ws + Vh + Uf) @ v_w                       [B, T]
    w    = softmax(e, axis=1)                             [B, T]
    ctx  = w @ encoding                                   [B, E]

Key kernel decompositions:
  * conv+U_w collapse:  Uf^T = G^T @ caw_shifted  with  G[k,a] = sum_l F_w[l,0,k] U_w[l,a]
    (caw_shifted[k, t] = caw[t + k - 15], zero padded; materialized host-side, im2col style)
  * Vh needs encoding with emb on partitions -> on-chip PE transposes of bf16 tiles
  * Vh and Uf accumulate into the same PSUM bank; ACT applies tanh with per-partition
    bias Ws[b] in one pass
  * energies via lhsT=th matvec -> e lands T-on-partitions, softmax without max
    subtraction (|e| <= sum|v| ~ 9, exp is safe in fp32)
  * ctx accumulated over T-chunks in PSUM with lhsT = normalized weights

Sharding: data-parallel over batch, 8 batches per core, params replicated.
"""

import sys
import types

import numpy as np
import ml_dtypes

B_FULL, T, E, A = 64, 2048, 768, 128
NCORES = 8
BS = B_FULL // NCORES          # 8 batches per core
EC = E // 128                  # 6 emb blocks
TC = T // 128                  # 16 T-chunks of 128
NTT = 4                        # T-tiles of 512 per batch
TTILE = 512
KS = 31
PAD = 15
LSTM = 1024

_CACHE = {}


def _install_ntff_shim():
    """Make trace=True work: bass_utils wants antenv.axon_hooks, which this
    image lacks; bridge to the ctypes hook from trn_agent_boot."""
    try:
        from antenv import axon_hooks  # noqa: F401
        return
    except ImportError:
        pass
    try:
        from trn_agent_boot.trn_boot import _ntff_profile_via_ctypes
        mod = types.ModuleType("antenv.axon_hooks")
        hook = _ntff_profile_via_ctypes("/opt/axon/libaxon_pjrt.so")
        mod.get_axon_ntff_profile_hook = lambda: hook
        mod.set_axon_ntff_profile_hook = lambda h: None
        sys.modules["antenv.axon_hooks"] = mod
    except Exception:
        pass


def _build_nc():
    import concourse.bass as bass
    import concourse.tile as tile
    from concourse import bacc, masks, mybir

    f32 = mybir.dt.float32
    bf16 = mybir.dt.bfloat16
    AF = mybir.ActivationFunctionType

    nc = bacc.Bacc("TRN2", target_bir_lowering=False, debug=False,
                   num_devices=NCORES)

    enc = nc.dram_tensor("enc", [BS, T, E], f32, kind="ExternalInput").ap()
    dsT = nc.dram_tensor("dsT", [LSTM, BS], f32, kind="ExternalInput").ap()
    caw_sh = nc.dram_tensor("caw_sh", [BS, KS, T], bf16, kind="ExternalInput").ap()
    W_w = nc.dram_tensor("W_w", [LSTM, A], f32, kind="ExternalInput").ap()
    W_b = nc.dram_tensor("W_b", [A, 1], f32, kind="ExternalInput").ap()
    Vw = nc.dram_tensor("Vw", [EC, 128, A], bf16, kind="ExternalInput").ap()
    G = nc.dram_tensor("G", [KS, A], bf16, kind="ExternalInput").ap()
    v_col = nc.dram_tensor("v_col", [A, 1], f32, kind="ExternalInput").ap()
    ones_col = nc.dram_tensor("ones_col", [128, 1], f32, kind="ExternalInput").ap()
    ones_row = nc.dram_tensor("ones_row", [1, 128], f32, kind="ExternalInput").ap()
    ctx_out = nc.dram_tensor("ctx_out", [BS, E], f32, kind="ExternalOutput").ap()
    w_out = nc.dram_tensor("w_out", [BS, TC, 128], f32, kind="ExternalOutput").ap()

    with tile.TileContext(nc) as tc:
        from contextlib import ExitStack
        with ExitStack() as ctx:
            const = ctx.enter_context(tc.tile_pool(name="const", bufs=1))

            vw_sb = const.tile([128, EC, A], bf16)
            nc.sync.dma_start(vw_sb[:], Vw.rearrange("c p m -> p c m"))
            g_sb = const.tile([KS, A], bf16)
            nc.sync.dma_start(g_sb[:], G)
            v_sb = const.tile([A, 1], f32)
            nc.sync.dma_start(v_sb[:], v_col)
            onec_sb = const.tile([128, 1], f32)
            nc.sync.dma_start(onec_sb[:], ones_col)
            oner_sb = const.tile([1, 128], f32)
            nc.sync.dma_start(oner_sb[:], ones_row)
            wb_sb = const.tile([A, 1], f32)
            nc.sync.dma_start(wb_sb[:], W_b)
            id_bf = const.tile([128, 128], bf16)
            masks.make_identity(nc, id_bf[:])
            id_f32 = const.tile([128, 128], f32)
            masks.make_identity(nc, id_f32[:])
            ws_sb = const.tile([A, BS], f32)

            # ---- Ws = W_w^T @ ds^T + W_b  (once) ----
            with tc.tile_pool(name="pre", bufs=1) as pre, \
                 tc.tile_pool(name="pre_ps", bufs=1, space="PSUM") as pre_ps:
                ww_sb = pre.tile([128, LSTM // 128, A], f32)
                nc.sync.dma_start(ww_sb[:], W_w.rearrange("(c p) m -> p c m", p=128))
                dst_sb = pre.tile([128, LSTM // 128, BS], f32)
                nc.sync.dma_start(dst_sb[:], dsT.rearrange("(c p) b -> p c b", p=128))
                ws_ps = pre_ps.tile([A, BS], f32)
                for c in range(LSTM // 128):
                    nc.tensor.matmul(ws_ps[:], ww_sb[:, c, :], dst_sb[:, c, :],
                                     start=(c == 0), stop=(c == LSTM // 128 - 1))
                nc.vector.tensor_scalar(out=ws_sb[:], in0=ws_ps[:],
                                        scalar1=wb_sb[:], scalar2=None,
                                        op0=mybir.AluOpType.add)

            enc_pool = ctx.enter_context(tc.tile_pool(name="encp", bufs=2))
            encT_pool = ctx.enter_context(tc.tile_pool(name="encT", bufs=2))
            th_pool = ctx.enter_context(tc.tile_pool(name="th", bufs=2))
            caw_pool = ctx.enter_context(tc.tile_pool(name="caw", bufs=2))
            sm_pool = ctx.enter_context(tc.tile_pool(name="sm", bufs=2))
            out_pool = ctx.enter_context(tc.tile_pool(name="outp", bufs=2))
            tr_ps = ctx.enter_context(tc.tile_pool(name="tr_ps", bufs=2, space="PSUM"))
            vh_ps = ctx.enter_context(tc.tile_pool(name="vh_ps", bufs=2, space="PSUM"))
            e_ps_pool = ctx.enter_context(tc.tile_pool(name="e_ps", bufs=1, space="PSUM"))
            ctx_ps_pool = ctx.enter_context(tc.tile_pool(name="ctx_ps", bufs=1, space="PSUM"))

            for b in range(BS):
                enc_sb = enc_pool.tile([128, TC, E], bf16, tag="enc")
                enc_b = enc[b].rearrange("(c p) e -> p c e", p=128)
                for q in range(NTT):
                    # SWDGE cast f32->bf16 during the HBM load
                    nc.gpsimd.dma_start(enc_sb[:, q * 4:(q + 1) * 4, :],
                                        enc_b[:, q * 4:(q + 1) * 4, :])
                caw_sb = caw_pool.tile([KS, T], bf16, tag="caw")
                nc.sync.dma_start(caw_sb[:], caw_sh[b])

                e_ps = e_ps_pool.tile([128, TC], f32, tag="e")

                for tt in range(NTT):
                    encT = encT_pool.tile([128, EC, TTILE], bf16, tag="encT")
                    for c in range(EC):
                        trp = tr_ps.tile([128, TTILE], bf16, tag="tr")
                        for s in range(4):
                            nc.tensor.transpose(
                                trp[:, s * 128:(s + 1) * 128],
                                enc_sb[:, tt * 4 + s, c * 128:(c + 1) * 128],
                                id_bf[:])
                        # alternate copy engine to split the PSUM->SBUF load
                        if c % 2 == 0:
                            nc.vector.tensor_copy(encT[:, c, :], trp[:])
                        else:
                            nc.scalar.copy(encT[:, c, :], trp[:])

                    vh = vh_ps.tile([128, TTILE], f32, tag="vh")
                    for c in range(EC):
                        nc.tensor.matmul(vh[:], vw_sb[:, c, :], encT[:, c, :],
                                         start=(c == 0), stop=False)
                    nc.tensor.matmul(vh[:], g_sb[:],
                                     caw_sb[:, tt * TTILE:(tt + 1) * TTILE],
                                     start=False, stop=True)

                    th = th_pool.tile([128, TTILE], f32, tag="th")
                    nc.scalar.activation(th[:], vh[:], AF.Tanh,
                                         bias=ws_sb[:, b:b + 1])

                    for s in range(4):
                        i = tt * 4 + s
                        nc.tensor.matmul(e_ps[:, i:i + 1],
                                         th[:, s * 128:(s + 1) * 128], v_sb[:],
                                         start=True, stop=True)

                # ---- softmax over T (no max subtraction; |e| < ~9) ----
                w_sb = sm_pool.tile([128, TC], f32, tag="w")
                zsum = sm_pool.tile([128, 1], f32, tag="zsum")
                nc.scalar.activation(w_sb[:], e_ps[:], AF.Exp, accum_out=zsum[:])
                z_ps = e_ps_pool.tile([1, 1], f32, tag="zq")
                nc.tensor.matmul(z_ps[:], onec_sb[:], zsum[:], start=True, stop=True)
                z_sb = sm_pool.tile([1, 1], f32, tag="zsb")
                nc.vector.tensor_copy(z_sb[:], z_ps[:])
                zb_ps = e_ps_pool.tile([128, 1], f32, tag="zq")
                nc.tensor.matmul(zb_ps[:], oner_sb[:], z_sb[:], start=True, stop=True)
                rz = sm_pool.tile([128, 1], f32, tag="rz")
                nc.vector.reciprocal(rz[:], zb_ps[:])
                nc.vector.tensor_scalar_mul(w_sb[:], w_sb[:], rz[:])

                # weights out: [128, TC] -> [TC, 128] so DRAM rows are contiguous
                wT_ps = tr_ps.tile([TC, 128], f32, tag="tr")
                nc.tensor.transpose(wT_ps[:], w_sb[:], id_f32[:])
                wT_sb = out_pool.tile([TC, 128], f32, tag="wT")
                nc.vector.tensor_copy(wT_sb[:], wT_ps[:])
                nc.sync.dma_start(w_out[b], wT_sb[:])

                # ---- ctx = sum_t w_t * enc[t, :]  (lhsT = w chunks, bf16) ----
                w_bf = sm_pool.tile([128, TC], bf16, tag="wbf")
                nc.vector.tensor_copy(w_bf[:], w_sb[:])
                ctx_a = ctx_ps_pool.tile([1, 512], f32, tag="ctx_a")
                ctx_b = ctx_ps_pool.tile([1, 256], f32, tag="ctx_b")
                for ch in range(TC):
                    nc.tensor.matmul(ctx_a[:], w_bf[:, ch:ch + 1],
                                     enc_sb[:, ch, 0:512],
                                     start=(ch == 0), stop=(ch == TC - 1))
                    nc.tensor.matmul(ctx_b[:], w_bf[:, ch:ch + 1],
                                     enc_sb[:, ch, 512:768],
                                     start=(ch == 0), stop=(ch == TC - 1))
                ctx_sb = out_pool.tile([1, E], f32, tag="ctx")
                nc.vector.tensor_copy(ctx_sb[:, 0:512], ctx_a[:])
                nc.vector.tensor_copy(ctx_sb[:, 512:768], ctx_b[:])
                nc.sync.dma_start(ctx_out[b], ctx_sb[:])

    nc.compile()
    return nc


def _get_nc():
    if "nc" not in _CACHE:
        _install_ntff_shim()
        _CACHE["nc"] = _build_nc()
    return _CACHE["nc"]


def _prep_inputs(decoder_state, encoding, cumulative_attention_weights,
                 W_w, W_b, V_w, U_w, F_w, v_w):
    bf = ml_dtypes.bfloat16
    f32 = np.float32
    ds = np.ascontiguousarray(np.asarray(decoder_state, f32))
    enc = np.ascontiguousarray(np.asarray(encoding, f32))
    caw = np.ascontiguousarray(np.asarray(cumulative_attention_weights, f32))
    W_w = np.asarray(W_w, f32)
    W_b = np.asarray(W_b, f32)
    V_w = np.asarray(V_w, f32)
    U_w = np.asarray(U_w, f32)
    F_w = np.asarray(F_w, f32)
    v_w = np.asarray(v_w, f32)

    # host-side weight packing / input marshalling
    dsT = np.ascontiguousarray(ds.T)                                  # [1024, 64]
    caw_pad = np.pad(caw, ((0, 0), (PAD, PAD)))
    idx = np.arange(T)[None, :] + np.arange(KS)[:, None]              # [31, T]
    caw_sh = np.ascontiguousarray(caw_pad[:, idx]).astype(bf)         # [64, 31, T]
    G = (F_w[:, 0, :].T @ U_w).astype(bf)                             # [31, 128]
    Vw_ch = np.ascontiguousarray(V_w.reshape(EC, 128, A)).astype(bf)  # [6, 128, 128]
    W_b_col = np.ascontiguousarray(W_b.reshape(A, 1))
    v_colv = np.ascontiguousarray(v_w.reshape(A, 1))
    ones_col = np.ones((128, 1), f32)
    ones_row = np.ones((1, 128), f32)

    in_maps = []
    for core in range(NCORES):
        sl = slice(core * BS, (core + 1) * BS)
        in_maps.append({
            "enc": enc[sl],
            "dsT": np.ascontiguousarray(dsT[:, sl]),
            "caw_sh": caw_sh[sl],
            "W_w": W_w,
            "W_b": W_b_col,
            "Vw": Vw_ch,
            "G": G,
            "v_col": v_colv,
            "ones_col": ones_col,
            "ones_row": ones_row,
        })
    return in_maps


def run_on_device(in_maps, trace=False, trace_kwargs=None):
    from concourse.bass_utils import run_bass_kernel_spmd
    nc = _get_nc()
    res = run_bass_kernel_spmd(nc, in_maps, list(range(NCORES)),
                               trace=trace, **(trace_kwargs or {}))
    ctx = np.concatenate([res.results[i]["ctx_out"] for i in range(NCORES)], axis=0)
    w = np.concatenate(
        [res.results[i]["w_out"].reshape(BS, T) for i in range(NCORES)], axis=0)
    return (ctx.astype(np.float32), w.astype(np.float32)), res


def kernel(**inputs):
    in_maps = _prep_inputs(**inputs)
    (ctx, w), _ = run_on_device(in_maps, trace=False)
    return (ctx, w)


# revision 8
# speedup vs baseline: 1.0514x; 1.0514x over previous
"""Trainium2 Bass kernel for LocationSensitiveAttention.

Math (B=64, T=2048, E=768, A=128, L=32, K=31):
    Ws   = decoder_state @ W_w + W_b                      [B, A]
    Vh   = encoding @ V_w                                 [B, T, A]
    loc  = conv1d(caw, F_w, pad=15)                       [B, L, T]
    Uf   = loc^T @ U_w                                    [B, T, A]
    e    = tanh(Ws + Vh + Uf) @ v_w                       [B, T]
    w    = softmax(e, axis=1)                             [B, T]
    ctx  = w @ encoding                                   [B, E]

Key kernel decompositions:
  * conv+U_w collapse:  Uf^T = G^T @ caw_shifted  with  G[k,a] = sum_l F_w[l,0,k] U_w[l,a]
    (caw_shifted[k, t] = caw[t + k - 15], zero padded; materialized host-side, im2col style)
  * Vh needs encoding with emb on partitions -> on-chip PE transposes of bf16 tiles
  * Vh and Uf accumulate into the same PSUM bank; ACT applies tanh with per-partition
    bias Ws[b] in one pass
  * energies via lhsT=th matvec -> e lands T-on-partitions, softmax without max
    subtraction (|e| <= sum|v| ~ 9, exp is safe in fp32)
  * ctx accumulated over T-chunks in PSUM with lhsT = normalized weights

Sharding: data-parallel over batch, 8 batches per core, params replicated.
"""

import sys
import types

import numpy as np
import ml_dtypes

B_FULL, T, E, A = 64, 2048, 768, 128
NCORES = 8
BS = B_FULL // NCORES          # 8 batches per core
EC = E // 128                  # 6 emb blocks
TC = T // 128                  # 16 T-chunks of 128
NTT = 4                        # T-tiles of 512 per batch
TTILE = 512
KS = 31
PAD = 15
LSTM = 1024

_CACHE = {}


def _install_ntff_shim():
    """Make trace=True work: bass_utils wants antenv.axon_hooks, which this
    image lacks; bridge to the ctypes hook from trn_agent_boot."""
    try:
        from antenv import axon_hooks  # noqa: F401
        return
    except ImportError:
        pass
    try:
        from trn_agent_boot.trn_boot import _ntff_profile_via_ctypes
        mod = types.ModuleType("antenv.axon_hooks")
        hook = _ntff_profile_via_ctypes("/opt/axon/libaxon_pjrt.so")
        mod.get_axon_ntff_profile_hook = lambda: hook
        mod.set_axon_ntff_profile_hook = lambda h: None
        sys.modules["antenv.axon_hooks"] = mod
    except Exception:
        pass


def _build_nc():
    import concourse.bass as bass
    import concourse.tile as tile
    from concourse import bacc, masks, mybir

    f32 = mybir.dt.float32
    bf16 = mybir.dt.bfloat16
    AF = mybir.ActivationFunctionType

    nc = bacc.Bacc("TRN2", target_bir_lowering=False, debug=False,
                   num_devices=NCORES)

    enc = nc.dram_tensor("enc", [BS, T, E], f32, kind="ExternalInput").ap()
    dsT = nc.dram_tensor("dsT", [LSTM, BS], f32, kind="ExternalInput").ap()
    caw_sh = nc.dram_tensor("caw_sh", [BS, KS, T], bf16, kind="ExternalInput").ap()
    W_w = nc.dram_tensor("W_w", [LSTM, A], f32, kind="ExternalInput").ap()
    W_b = nc.dram_tensor("W_b", [A, 1], f32, kind="ExternalInput").ap()
    Vw = nc.dram_tensor("Vw", [EC, 128, A], bf16, kind="ExternalInput").ap()
    G = nc.dram_tensor("G", [KS, A], bf16, kind="ExternalInput").ap()
    v_col = nc.dram_tensor("v_col", [A, 1], bf16, kind="ExternalInput").ap()
    ones_col = nc.dram_tensor("ones_col", [128, 1], f32, kind="ExternalInput").ap()
    ones_row = nc.dram_tensor("ones_row", [1, 128], f32, kind="ExternalInput").ap()
    ctx_out = nc.dram_tensor("ctx_out", [BS, E], f32, kind="ExternalOutput").ap()
    w_out = nc.dram_tensor("w_out", [BS, TC, 128], f32, kind="ExternalOutput").ap()

    with tile.TileContext(nc) as tc:
        from contextlib import ExitStack
        with ExitStack() as ctx:
            const = ctx.enter_context(tc.tile_pool(name="const", bufs=1))

            vw_sb = const.tile([128, EC, A], bf16)
            nc.sync.dma_start(vw_sb[:], Vw.rearrange("c p m -> p c m"))
            g_sb = const.tile([KS, A], bf16)
            nc.sync.dma_start(g_sb[:], G)
            v_sb = const.tile([A, 1], bf16)
            nc.sync.dma_start(v_sb[:], v_col)
            onec_sb = const.tile([128, 1], f32)
            nc.sync.dma_start(onec_sb[:], ones_col)
            oner_sb = const.tile([1, 128], f32)
            nc.sync.dma_start(oner_sb[:], ones_row)
            wb_sb = const.tile([A, 1], f32)
            nc.sync.dma_start(wb_sb[:], W_b)
            id_bf = const.tile([128, 128], bf16)
            masks.make_identity(nc, id_bf[:])
            id_f32 = const.tile([128, 128], f32)
            masks.make_identity(nc, id_f32[:])
            ws_sb = const.tile([A, BS], f32)

            # ---- Ws = W_w^T @ ds^T + W_b  (once) ----
            with tc.tile_pool(name="pre", bufs=1) as pre, \
                 tc.tile_pool(name="pre_ps", bufs=1, space="PSUM") as pre_ps:
                ww_sb = pre.tile([128, LSTM // 128, A], f32)
                nc.sync.dma_start(ww_sb[:], W_w.rearrange("(c p) m -> p c m", p=128))
                dst_sb = pre.tile([128, LSTM // 128, BS], f32)
                nc.sync.dma_start(dst_sb[:], dsT.rearrange("(c p) b -> p c b", p=128))
                ws_ps = pre_ps.tile([A, BS], f32)
                for c in range(LSTM // 128):
                    nc.tensor.matmul(ws_ps[:], ww_sb[:, c, :], dst_sb[:, c, :],
                                     start=(c == 0), stop=(c == LSTM // 128 - 1))
                nc.vector.tensor_scalar(out=ws_sb[:], in0=ws_ps[:],
                                        scalar1=wb_sb[:], scalar2=None,
                                        op0=mybir.AluOpType.add)

            enc_pool = ctx.enter_context(tc.tile_pool(name="encp", bufs=3))
            encT_pool = ctx.enter_context(tc.tile_pool(name="encT", bufs=2))
            th_pool = ctx.enter_context(tc.tile_pool(name="th", bufs=2))
            caw_pool = ctx.enter_context(tc.tile_pool(name="caw", bufs=2))
            sm_pool = ctx.enter_context(tc.tile_pool(name="sm", bufs=2))
            out_pool = ctx.enter_context(tc.tile_pool(name="outp", bufs=2))
            tr_ps = ctx.enter_context(tc.tile_pool(name="tr_ps", bufs=2, space="PSUM"))
            vh_ps = ctx.enter_context(tc.tile_pool(name="vh_ps", bufs=2, space="PSUM"))
            e_ps_pool = ctx.enter_context(tc.tile_pool(name="e_ps", bufs=1, space="PSUM"))
            ctx_ps_pool = ctx.enter_context(tc.tile_pool(name="ctx_ps", bufs=1, space="PSUM"))

            for b in range(BS):
                enc_sb = enc_pool.tile([128, TC, E], bf16, tag="enc")
                enc_b = enc[b].rearrange("(c p) e -> p c e", p=128)
                for q in range(NTT):
                    # SWDGE cast f32->bf16 during the HBM load
                    nc.gpsimd.dma_start(enc_sb[:, q * 4:(q + 1) * 4, :],
                                        enc_b[:, q * 4:(q + 1) * 4, :])
                caw_sb = caw_pool.tile([KS, T], bf16, tag="caw")
                nc.sync.dma_start(caw_sb[:], caw_sh[b])

                e_ps = e_ps_pool.tile([128, TC], f32, tag="e")

                for tt in range(NTT):
                    encT = encT_pool.tile([128, EC, TTILE], bf16, tag="encT")
                    for c in range(EC):
                        trp = tr_ps.tile([128, TTILE], f32, tag="tr")
                        for s in range(4):
                            # transpose as a REGULAR matmul (identity rhs):
                            # pipelines ~2x faster than PE transpose_mode
                            nc.tensor.matmul(
                                trp[:, s * 128:(s + 1) * 128],
                                enc_sb[:, tt * 4 + s, c * 128:(c + 1) * 128],
                                id_bf[:], start=True, stop=True)
                        # alternate copy engine to split the PSUM->SBUF load
                        if c % 2 == 0:
                            nc.vector.tensor_copy(encT[:, c, :], trp[:])
                        else:
                            nc.scalar.copy(encT[:, c, :], trp[:])

                    vh = vh_ps.tile([128, TTILE], f32, tag="vh")
                    for c in range(EC):
                        nc.tensor.matmul(vh[:], vw_sb[:, c, :], encT[:, c, :],
                                         start=(c == 0), stop=False)
                    nc.tensor.matmul(vh[:], g_sb[:],
                                     caw_sb[:, tt * TTILE:(tt + 1) * TTILE],
                                     start=False, stop=True)

                    th = th_pool.tile([128, TTILE], bf16, tag="th")
                    nc.scalar.activation(th[:], vh[:], AF.Tanh,
                                         bias=ws_sb[:, b:b + 1])

                    for s in range(4):
                        i = tt * 4 + s
                        nc.tensor.matmul(e_ps[:, i:i + 1],
                                         th[:, s * 128:(s + 1) * 128], v_sb[:],
                                         start=True, stop=True)

                # ---- softmax over T (no max subtraction; |e| < ~9) ----
                w_sb = sm_pool.tile([128, TC], f32, tag="w")
                zsum = sm_pool.tile([128, 1], f32, tag="zsum")
                nc.scalar.activation(w_sb[:], e_ps[:], AF.Exp, accum_out=zsum[:])
                z_ps = e_ps_pool.tile([1, 1], f32, tag="zq")
                nc.tensor.matmul(z_ps[:], onec_sb[:], zsum[:], start=True, stop=True)
                z_sb = sm_pool.tile([1, 1], f32, tag="zsb")
                nc.vector.tensor_copy(z_sb[:], z_ps[:])
                zb_ps = e_ps_pool.tile([128, 1], f32, tag="zq")
                nc.tensor.matmul(zb_ps[:], oner_sb[:], z_sb[:], start=True, stop=True)
                rz = sm_pool.tile([128, 1], f32, tag="rz")
                nc.vector.reciprocal(rz[:], zb_ps[:])
                nc.vector.tensor_scalar_mul(w_sb[:], w_sb[:], rz[:])

                # weights out: [128, TC] -> [TC, 128] so DRAM rows are contiguous
                wT_ps = tr_ps.tile([TC, 128], f32, tag="tr")
                nc.tensor.transpose(wT_ps[:], w_sb[:], id_f32[:])
                wT_sb = out_pool.tile([TC, 128], f32, tag="wT")
                nc.vector.tensor_copy(wT_sb[:], wT_ps[:])
                nc.sync.dma_start(w_out[b], wT_sb[:])

                # ---- ctx = sum_t w_t * enc[t, :]  (lhsT = w chunks, bf16) ----
                w_bf = sm_pool.tile([128, TC], bf16, tag="wbf")
                nc.vector.tensor_copy(w_bf[:], w_sb[:])
                ctx_a = ctx_ps_pool.tile([1, 512], f32, tag="ctx_a")
                ctx_b = ctx_ps_pool.tile([1, 256], f32, tag="ctx_b")
                for ch in range(TC):
                    nc.tensor.matmul(ctx_a[:], w_bf[:, ch:ch + 1],
                                     enc_sb[:, ch, 0:512],
                                     start=(ch == 0), stop=(ch == TC - 1))
                    nc.tensor.matmul(ctx_b[:], w_bf[:, ch:ch + 1],
                                     enc_sb[:, ch, 512:768],
                                     start=(ch == 0), stop=(ch == TC - 1))
                ctx_sb = out_pool.tile([1, E], f32, tag="ctx")
                nc.vector.tensor_copy(ctx_sb[:, 0:512], ctx_a[:])
                nc.vector.tensor_copy(ctx_sb[:, 512:768], ctx_b[:])
                nc.sync.dma_start(ctx_out[b], ctx_sb[:])

    nc.compile()
    return nc


def _get_nc():
    if "nc" not in _CACHE:
        _install_ntff_shim()
        _CACHE["nc"] = _build_nc()
    return _CACHE["nc"]


def _prep_inputs(decoder_state, encoding, cumulative_attention_weights,
                 W_w, W_b, V_w, U_w, F_w, v_w):
    bf = ml_dtypes.bfloat16
    f32 = np.float32
    ds = np.ascontiguousarray(np.asarray(decoder_state, f32))
    enc = np.ascontiguousarray(np.asarray(encoding, f32))
    caw = np.ascontiguousarray(np.asarray(cumulative_attention_weights, f32))
    W_w = np.asarray(W_w, f32)
    W_b = np.asarray(W_b, f32)
    V_w = np.asarray(V_w, f32)
    U_w = np.asarray(U_w, f32)
    F_w = np.asarray(F_w, f32)
    v_w = np.asarray(v_w, f32)

    # host-side weight packing / input marshalling
    dsT = np.ascontiguousarray(ds.T)                                  # [1024, 64]
    caw_pad = np.pad(caw, ((0, 0), (PAD, PAD)))
    idx = np.arange(T)[None, :] + np.arange(KS)[:, None]              # [31, T]
    caw_sh = np.ascontiguousarray(caw_pad[:, idx]).astype(bf)         # [64, 31, T]
    G = (F_w[:, 0, :].T @ U_w).astype(bf)                             # [31, 128]
    Vw_ch = np.ascontiguousarray(V_w.reshape(EC, 128, A)).astype(bf)  # [6, 128, 128]
    W_b_col = np.ascontiguousarray(W_b.reshape(A, 1))
    v_colv = np.ascontiguousarray(v_w.reshape(A, 1)).astype(bf)
    ones_col = np.ones((128, 1), f32)
    ones_row = np.ones((1, 128), f32)

    in_maps = []
    for core in range(NCORES):
        sl = slice(core * BS, (core + 1) * BS)
        in_maps.append({
            "enc": enc[sl],
            "dsT": np.ascontiguousarray(dsT[:, sl]),
            "caw_sh": caw_sh[sl],
            "W_w": W_w,
            "W_b": W_b_col,
            "Vw": Vw_ch,
            "G": G,
            "v_col": v_colv,
            "ones_col": ones_col,
            "ones_row": ones_row,
        })
    return in_maps


def run_on_device(in_maps, trace=False, trace_kwargs=None):
    from concourse.bass_utils import run_bass_kernel_spmd
    nc = _get_nc()
    res = run_bass_kernel_spmd(nc, in_maps, list(range(NCORES)),
                               trace=trace, **(trace_kwargs or {}))
    ctx = np.concatenate([res.results[i]["ctx_out"] for i in range(NCORES)], axis=0)
    w = np.concatenate(
        [res.results[i]["w_out"].reshape(BS, T) for i in range(NCORES)], axis=0)
    return (ctx.astype(np.float32), w.astype(np.float32)), res


def kernel(**inputs):
    in_maps = _prep_inputs(**inputs)
    (ctx, w), _ = run_on_device(in_maps, trace=False)
    return (ctx, w)
